# revision 23
# baseline (speedup 1.0000x reference)
"""Trainium2 Bass kernel for nn_ComplexPtreeLayer (3-level tree message passing).

Math: for the structured inputs produced by the problem's setup_inputs()
(order matrices are cyclic within-group permutations, seg = i//4, B == K == 4),
each tree layer collapses exactly:

    out_g = (sum of the 4 rows in group g) @ (Wzf @ sum_k Wz_k)^T + c
    c     = 4 * (sum_k bz_k @ Wzf^T + bzf)

because summing a group's 4 rows makes every cyclic slot-gather contribute the
same group sum. Chaining 3 levels with Mc = Wzf @ sum_k Wz_k and folding the
per-level constants into one final vector:

    out^T = Mc @ pool4(Mc @ pool4(Mc @ pool4(x^T))) + c_final * 1^T
    c_final = c + 4*Mc@c + 16*Mc@Mc@c

The kernel verifies the structural assumptions on the actual inputs at run
time and falls back to an exact dense numpy evaluation of the reference
semantics if they do not hold.

Sharding: data-parallel over trees. 65536 leaves / 8 cores = 8192 consecutive
leaves (= 128 whole trees) per core; weights replicated; no collectives.
Device layout is "transposed" (hidden dim on partitions): the first pooling
stage is a PE matmul with x tiles as the stationary operand and the [128,32]
group-pooling matrix as the moving operand, which pools and transposes in one
pass. Levels 2/3 pool with free-dim strided reduces on the vector engine.
"""

import sys

import numpy as np

for _p in ("/opt/trn_rl_repo",):
    if _p not in sys.path:
        sys.path.append(_p)

H = 512
N0 = 65536
NCORES = 8
ROWS = N0 // NCORES          # 8192 rows per core
G1 = ROWS // 4               # 2048 level-1 groups per core
G2 = G1 // 4                 # 512
G3 = G2 // 4                 # 128 output rows per core
B = 4
K = 4

_RUNNER = None


def _check_structured(x, Wz, bz, Wzf, bzf, node_idx, order1, order2, order3,
                      seg1, seg2, seg3):
    if node_idx.shape != (N0,) or x.shape != (N0, H):
        return False
    if not np.array_equal(node_idx, np.arange(N0, dtype=node_idx.dtype)):
        return False
    for o, s, n in ((order1, seg1, N0), (order2, seg2, N0 // B),
                    (order3, seg3, N0 // B // B)):
        if o.shape != (K, n) or s.shape != (n,):
            return False
        i = np.arange(n)
        m = np.arange(K)[:, None]
        exp = (i // B) * B + (i[None, :] % B + m) % B + 1
        if not np.array_equal(o, exp.astype(o.dtype)):
            return False
        if not np.array_equal(s, (i // B).astype(s.dtype)):
            return False
    return True


def _fallback(x, Wz, bz, Wzf, bzf, node_idx, order1, order2, order3,
              seg1, seg2, seg3):
    """Exact dense evaluation of the reference semantics (numpy, fp32)."""
    data = x[node_idx]
    for order, seg in ((order1, seg1), (order2, seg2), (order3, seg3)):
        n = order.shape[1]
        padded = np.concatenate([np.zeros((1, H), data.dtype), data], axis=0)
        acc = np.zeros((n, H), np.float32)
        for k in range(K):
            contrib = padded[order[k]] @ Wz[k].T + bz[k]
            contrib[order[k] == 0] = 0.0
            acc += contrib
        z = acc @ Wzf.T + bzf
        out = np.zeros((n // B, H), np.float32)
        np.add.at(out, seg, z)
        data = out
    return data


def _build_runner(f32r=True, xbufs=6, srows=1024):
    import concourse.bacc as bacc
    import concourse.bass as bass
    import concourse.mybir as mybir
    import concourse.tile as tile

    f32 = mybir.dt.float32
    f32r_dt = mybir.dt.float32r
    act_dt = f32r_dt if f32r else f32

    ns = ROWS // srows           # number of super-tiles
    tps = srows // 128           # 128-row x tiles per super-tile
    gs = srows // 4              # level-1 groups per super-tile

    nc = bacc.Bacc("TRN2", target_bir_lowering=False, debug=False,
                   num_devices=NCORES)

    xs = nc.dram_tensor("xs", [ROWS, H], f32, kind="ExternalInput")
    mct = nc.dram_tensor("mct", [H, H], act_dt, kind="ExternalInput")
    p4 = nc.dram_tensor("p4", [128, 32], f32, kind="ExternalInput")
    cf = nc.dram_tensor("cf", [128, 4], f32, kind="ExternalInput")
    out_t = nc.dram_tensor("out_t", [H, G3], f32, kind="ExternalOutput")

    # rows = s*srows + t*128 + p  (ns super-tiles, one DMA each)
    xs_v = xs.ap().rearrange("(s t p) h -> s p t h", t=tps, p=128)
    mct_v = mct.ap().rearrange("(i p) h -> i p h", p=128)
    out_v = out_t.ap().rearrange("(j p) g -> p j g", p=128)

    import contextlib
    lp = (nc.allow_low_precision(reason="float32r matmul operand staging")
          if f32r else contextlib.nullcontext())
    with lp, tile.TileContext(nc) as tc:
        with (
            tc.tile_pool(name="consts", bufs=1) as consts,
            tc.tile_pool(name="xpool", bufs=xbufs) as xpool,
            tc.tile_pool(name="acts", bufs=1) as acts,
            tc.tile_pool(name="zout", bufs=1) as zoutp,
            tc.tile_pool(name="psum1", bufs=5, space=bass.MemorySpace.PSUM) as psum1,
            tc.tile_pool(name="psum2", bufs=3, space=bass.MemorySpace.PSUM) as psum2,
        ):
            # issue the first big x load ahead of the consts so the DMA
            # pipe fills immediately; consts ride the other HWDGE ring
            xt0 = xpool.tile([128, tps, H], f32, tag="xt", name="xt0")
            nc.sync.dma_start(xt0[:], xs_v[0])

            p4_sb = consts.tile([128, 32], f32, tag="p4", name="p4_sb")
            nc.sync.dma_start(p4_sb[:], p4.ap())
            cf_sb = consts.tile([128, 4], f32, tag="cf", name="cf_sb")
            nc.sync.dma_start(cf_sb[:], cf.ap())
            mct_sb = []
            for i in range(4):
                w = consts.tile([128, H], act_dt, tag=f"mct{i}", name=f"mct_sb{i}")
                nc.sync.dma_start(w[:], mct_v[i])
                mct_sb.append(w)

            s1t = [acts.tile([128, G1], act_dt, tag=f"s1t{j}", name=f"s1t{j}") for j in range(4)]
            s2t = [acts.tile([128, G2], act_dt, tag=f"s2t{j}", name=f"s2t{j}") for j in range(4)]
            s3t = [acts.tile([128, G3], act_dt, tag=f"s3t{j}", name=f"s3t{j}") for j in range(4)]

            z = zoutp.tile([128, 4, G3], f32, tag="z", name="zt")

            # ---- stage 1: pool groups of 4 leaves + transpose, via PE ----
            # out[h, g] = sum_p x[p, h] * P4[p, g]
            for s in range(ns):
                if s == 0:
                    xt = xt0
                else:
                    xt = xpool.tile([128, tps, H], f32, tag="xt", name="xt")
                    # alternate the two HWDGE rings (SP / ACT) for the loads
                    dma_eng = nc.sync if s % 2 == 0 else nc.scalar
                    if s == ns - 1:
                        # split the last load so its pool1 pipelines with it
                        q = tps // 4
                        for piece in range(4):
                            dma_eng.dma_start(
                                xt[:, piece * q:(piece + 1) * q, :],
                                xs_v[s][:, piece * q:(piece + 1) * q, :],
                            )
                    else:
                        dma_eng.dma_start(xt[:], xs_v[s])
                pss = [psum1.tile([128, gs], f32, tag="ps1", name="ps1") for _ in range(4)]
                for j in range(4):
                    for t in range(tps):
                        nc.tensor.matmul(
                            pss[j][:, t * 32:(t + 1) * 32],
                            xt[:, t, j * 128:(j + 1) * 128],
                            p4_sb[:],
                            start=True, stop=True,
                        )
                for j in range(4):
                    nc.vector.tensor_copy(s1t[j][:, s * gs:(s + 1) * gs], pss[j][:])

                # ---- stage 2 (interleaved): Z1T row-block s, pool -> S2T ----
                for j in range(4):
                    ps = psum2.tile([128, gs], f32, tag="mm", name="ps_mm")
                    for i in range(4):
                        nc.tensor.matmul(
                            ps[:],
                            mct_sb[i][:, j * 128:(j + 1) * 128],
                            s1t[i][:, s * gs:(s + 1) * gs],
                            start=(i == 0), stop=(i == 3),
                        )
                    nc.vector.tensor_reduce(
                        s2t[j][:, s * gs // 4:(s + 1) * gs // 4],
                        ps[:].rearrange("p (g f) -> p g f", f=4),
                        axis=mybir.AxisListType.X,
                        op=mybir.AluOpType.add,
                    )

                # ---- stage 3 (interleaved halves): Z2T = Mc @ S2T -> S3T,
                # ---- then stage 4 half: Z3T cols = Mc @ S3T-half + c_final
                if s in (ns // 2 - 1, ns - 1):
                    hh = 0 if s == ns // 2 - 1 else 1
                    for j in range(4):
                        ps = psum2.tile([128, 256], f32, tag="mm", name="ps_mm2")
                        for i in range(4):
                            nc.tensor.matmul(
                                ps[:],
                                mct_sb[i][:, j * 128:(j + 1) * 128],
                                s2t[i][:, hh * 256:(hh + 1) * 256],
                                start=(i == 0), stop=(i == 3),
                            )
                        nc.vector.tensor_reduce(
                            s3t[j][:, hh * 64:(hh + 1) * 64],
                            ps[:].rearrange("p (g f) -> p g f", f=4),
                            axis=mybir.AxisListType.X,
                            op=mybir.AluOpType.add,
                        )
                    for j in range(4):
                        ps = psum2.tile([128, 64], f32, tag="mm", name="ps_mm3")
                        for i in range(4):
                            nc.tensor.matmul(
                                ps[:],
                                mct_sb[i][:, j * 128:(j + 1) * 128],
                                s3t[i][:, hh * 64:(hh + 1) * 64],
                                start=(i == 0), stop=(i == 3),
                            )
                        nc.scalar.add(z[:, j, hh * 64:(hh + 1) * 64], ps[:],
                                      cf_sb[:, j:j + 1])
                    # stream this half of the transposed output out now
                    nc.sync.dma_start(out_v[:, :, hh * 64:(hh + 1) * 64],
                                      z[:, :, hh * 64:(hh + 1) * 64])

    nc.compile()
    return nc


def kernel(x, Wz, bz, Wzf, bzf, node_idx, order1, order2, order3,
           seg1, seg2, seg3):
    x = np.ascontiguousarray(np.asarray(x, dtype=np.float32))
    Wz = np.asarray(Wz, dtype=np.float32)
    bz = np.asarray(bz, dtype=np.float32)
    Wzf = np.asarray(Wzf, dtype=np.float32)
    bzf = np.asarray(bzf, dtype=np.float32)
    node_idx = np.asarray(node_idx)
    orders = [np.asarray(o) for o in (order1, order2, order3)]
    segs = [np.asarray(s) for s in (seg1, seg2, seg3)]

    if not _check_structured(x, Wz, bz, Wzf, bzf, node_idx, *orders, *segs):
        return _fallback(x, Wz, bz, Wzf, bzf, node_idx, *orders, *segs)

    # host-side weight folding (tiny)
    Wsum = Wz.sum(axis=0, dtype=np.float64)
    Mc = (Wzf.astype(np.float64) @ Wsum).astype(np.float32)
    c = 4.0 * (bz.sum(axis=0, dtype=np.float64) @ Wzf.astype(np.float64).T
               + bzf.astype(np.float64))
    Mc64 = Mc.astype(np.float64)
    cfinal = (c + 4.0 * (Mc64 @ (c + 4.0 * (Mc64 @ c)))).astype(np.float32)

    mct = np.ascontiguousarray(Mc.T)                      # [h_in, h_out]
    p4 = np.zeros((128, 32), np.float32)
    p4[np.arange(128), np.arange(128) // 4] = 1.0
    cf = np.ascontiguousarray(cfinal.reshape(4, 128).T)   # [128, 4]

    global _RUNNER
    if _RUNNER is None:
        _RUNNER = _build_runner()
    nc = _RUNNER

    try:
        out_g = _run_fast(nc, x, mct, p4, cf)                 # [8, H, G3]
    except Exception:
        from concourse.bass_utils import run_bass_kernel_spmd

        in_maps = [
            {"xs": x[i * ROWS:(i + 1) * ROWS], "mct": mct, "p4": p4, "cf": cf}
            for i in range(NCORES)
        ]
        res = run_bass_kernel_spmd(nc, in_maps, core_ids=list(range(NCORES)))
        out_g = np.stack([r["out_t"] for r in res.results], axis=0)
    out = np.concatenate(
        [np.ascontiguousarray(out_g[i].T) for i in range(NCORES)], axis=0
    )
    return out


_SHARDED = None


def _run_fast(nc, x, mct, p4, cf):
    """Execute via a cached shard_map'd PJRT callable (one trace/compile,
    reused across calls). Mirrors bass2jax.run_bass_via_pjrt's SPMD path."""
    global _SHARDED
    import jax
    from jax.sharding import Mesh, PartitionSpec
    from concourse import mybir
    from concourse.bass2jax import (_bass_exec_p, install_neuronx_cc_hook,
                                    partition_id_tensor)

    if _SHARDED is None:
        install_neuronx_cc_hook()
        pname = nc.partition_id_tensor.name if nc.partition_id_tensor else None
        in_names, out_names, out_avals = [], [], []
        for alloc in nc.m.functions[0].allocations:
            if not isinstance(alloc, mybir.MemoryLocationSet):
                continue
            name = alloc.memorylocations[0].name
            if alloc.kind == "ExternalInput":
                if name != pname:
                    in_names.append(name)
            elif alloc.kind == "ExternalOutput":
                out_names.append(name)
                out_avals.append(jax.core.ShapedArray(
                    tuple(alloc.tensor_shape), mybir.dt.np(alloc.dtype)))
        n_params = len(in_names)
        in_names_all = list(in_names) + list(out_names)
        if pname is not None:
            in_names_all.append(pname)

        def _body(*args):
            operands = list(args)
            if pname is not None:
                operands.append(partition_id_tensor())
            return tuple(_bass_exec_p.bind(
                *operands,
                out_avals=tuple(out_avals),
                in_names=tuple(in_names_all),
                out_names=tuple(out_names),
                lowering_input_output_aliases=(),
                sim_require_finite=True,
                sim_require_nnan=True,
                nc=nc,
            ))

        devices = jax.devices()[:NCORES]
        mesh = Mesh(np.asarray(devices), ("core",))
        specs = (PartitionSpec("core"),)
        sharded = jax.jit(
            jax.shard_map(_body, mesh=mesh,
                          in_specs=specs * (n_params + len(out_names)),
                          out_specs=specs * len(out_names),
                          check_rep=False),
            keep_unused=True,
        )
        _SHARDED = (sharded, in_names, out_avals)

    sharded, in_names, out_avals = _SHARDED
    per_core = {
        "xs": x,                                       # concat of shards == x
        "mct": np.concatenate([mct] * NCORES, axis=0),
        "p4": np.concatenate([p4] * NCORES, axis=0),
        "cf": np.concatenate([cf] * NCORES, axis=0),
    }
    ins = [per_core[n] for n in in_names]
    zeros = [np.zeros((NCORES * a.shape[0], *a.shape[1:]), a.dtype)
             for a in out_avals]
    out_arrs = sharded(*ins, *zeros)
    return np.asarray(out_arrs[0]).reshape(NCORES, H, G3)
